# revision 1
# baseline (speedup 1.0000x reference)
"""Trainium2 Bass kernel for a GNN message-passing layer.

Strategy (node-range sharding, no collectives):
  - 8 cores, each owns 12500 destination nodes (98 windows of 128 nodes).
  - Host sorts edges by (core, dst-window, src%4), pads each (window, group)
    to 128-edge tiles (tile counts = max over cores, derived from data).
  - x[src] per edge: one dma_gather(transpose=True) per window from a
    quad-packed bf16 table xq = x.reshape(N/4, 256).  Output is feature-major
    [128, 2, S_w]: node (src%4) at partitions 64*(g%2), block g//2.  Edge
    tiles are grouped by g so slices are static.
  - x[dst] contribution: P2_win = x_win @ W1b per window (no gather), then
    expansion matmul  h += P2_win^T @ sel_T  with one-hot sel_T built on DVE.
  - scatter-add: agg^T[f, n] += msg^T @ sel accumulated in PSUM per window.
  - node MLP fused per window; output written once.

All matmuls bf16 (f32 PSUM accumulate).
"""

import numpy as np
import ml_dtypes

P = 128
H = 64
ED = 16
N_CORES = 8

# Native Silu on the ACT engine (HW supports it; CoreSim does not --
# sim tests flip this to False to use sigmoid+mul instead).
USE_NATIVE_SILU = True
USE_REG_COUNTS = False
SEL3D = False


# ---------------------------------------------------------------- host prep

def _prep(x, edge_index, edge_attr):
    """Sort/pad edges into the per-core slot layout; build device arrays."""
    n_nodes = x.shape[0]
    npc = n_nodes // N_CORES              # nodes per core
    nw = (npc + P - 1) // P               # windows per core
    npc_pad = nw * P

    src = edge_index[0].astype(np.int64)
    dst = edge_index[1].astype(np.int64)
    e = src.shape[0]

    core = dst // npc
    rem = dst - core * npc
    wl = rem // P                         # window within core
    dloc = rem - wl * P                   # dst index within window
    grp = src & 3

    # sort edges by (core, window, group); stable order inside
    key = ((core * nw + wl) * 4 + grp).astype(np.int64)
    order = np.argsort(key, kind="stable")
    key_s = key[order]
    src_s = src[order]
    dloc_s = dloc[order]

    counts = np.bincount(key_s, minlength=N_CORES * nw * 4)
    counts = counts.reshape(N_CORES, nw, 4)
    # tiles per (window, group): max over cores so one NEFF fits all cores
    twg = (counts.max(axis=0) + P - 1) // P          # [nw, 4]
    sw = twg.sum(axis=1) * P                         # slots per window [nw]
    if sw.max() == 0:
        sw[:] = 0
    e_pad = int(sw.sum())
    assert e_pad % 256 == 0 or True
    # slot base of each (w, g) block within the padded layout
    base_wg = np.zeros((nw, 4), dtype=np.int64)
    flat_bases = np.cumsum(np.concatenate([[0], (twg * P).reshape(-1)[:-1]]))
    base_wg[:, :] = flat_bases.reshape(nw, 4)

    # per-core slot arrays. Pad src idx: -1 (skipped) with reg-count
    # gathers, else 0 (valid dummy; sel column is 0 so it contributes nothing)
    srcq_slots = np.full((N_CORES, e_pad), -1 if USE_REG_COUNTS else 0,
                         dtype=np.int16)
    dloc_slots = np.full((N_CORES, e_pad), -1.0, dtype=np.float32)
    attr_slots = np.zeros((N_CORES, e_pad, ED), dtype=np.float32)

    # rank of each sorted edge within its (core,w,g) block
    starts = np.concatenate([[0], np.cumsum(counts.reshape(-1))[:-1]])
    rank = np.arange(e, dtype=np.int64) - starts[key_s]
    core_s = key_s // (nw * 4)
    wg_s = key_s - core_s * (nw * 4)     # (w*4+g) index
    slot = base_wg.reshape(-1)[wg_s] + rank

    srcq_slots[core_s, slot] = (src_s >> 2).astype(np.int16)
    dloc_slots[core_s, slot] = dloc_s.astype(np.float32)
    attr_slots[core_s, slot, :] = edge_attr[order]

    # per-core real counts per (w, g); ensure >= 1 valid idx per active block
    cnt = counts.reshape(N_CORES, nw * 4).astype(np.int32)
    active = (twg.reshape(-1) > 0)
    for c in range(N_CORES):
        for wg in np.nonzero(active & (cnt[c] == 0))[0]:
            srcq_slots[c, base_wg.reshape(-1)[wg]] = 0
    cnt = np.maximum(cnt, active.astype(np.int32)[None, :])
    cmin = cnt.min(axis=0).reshape(nw, 4)   # stale-region start per (w, g)

    bf16 = ml_dtypes.bfloat16

    # idx arrays for dma_gather: per window wrap-16 + replicate to 128 parts
    qidx = np.zeros((N_CORES, P, e_pad // 16), dtype=np.int16)
    col = 0
    for w in range(nw):
        s_w = int(sw[w])
        if s_w == 0:
            continue
        seg = srcq_slots[:, col * 16:col * 16 + s_w]           # [C, s_w]
        wrapped = seg.reshape(N_CORES, s_w // 16, 16).transpose(0, 2, 1)
        qidx[:, :, col:col + s_w // 16] = np.tile(wrapped, (1, 8, 1))
        col += s_w // 16

    dstc = np.ascontiguousarray(
        dloc_slots.reshape(N_CORES, e_pad // P, P).transpose(0, 2, 1)
    ).astype(bf16)                                             # [C,128,T]
    dstr_full = np.broadcast_to(
        dloc_slots[:, None, :], (N_CORES, P, e_pad)
    )
    dstr_full = np.ascontiguousarray(dstr_full).astype(bf16)   # [C,128,E]
    attrT = np.ascontiguousarray(
        attr_slots.transpose(0, 2, 1)
    ).astype(bf16)                                             # [C,16,E]

    # node tables
    xq = np.ascontiguousarray(x.reshape(n_nodes // 4, 256)).astype(bf16)
    xT = x.T.astype(bf16)                                      # [64, N]
    xTc = np.zeros((N_CORES, H, npc_pad), dtype=bf16)
    for c in range(N_CORES):
        xTc[c, :, :npc] = xT[:, c * npc:(c + 1) * npc]

    struct = {
        "nw": nw, "npc": npc, "npc_pad": npc_pad, "e_pad": e_pad,
        "twg": twg, "sw": sw, "cmin": cmin,
    }
    arrays = {
        "qidx": qidx, "dstc": dstc, "dstr": dstr_full, "attrT": attrT,
        "xq": xq, "xTc": xTc, "cnt": cnt,
    }
    return struct, arrays


def _prep_consts(W1, b1, W2, b2, W3, b3):
    bf16 = ml_dtypes.bfloat16
    w1a = W1[0:H, :]                     # src part      [64, 64]
    w1b = W1[H:2 * H, :]                 # dst part      [64, 64]
    w1c = W1[2 * H:2 * H + ED, :]        # attr part     [16, 64]
    w1a2 = np.concatenate([w1a, w1a], axis=0).astype(bf16)   # [128, 64]
    consts = {
        "w1a2": w1a2,
        "w1b": w1b.astype(bf16),
        "w1c": w1c.astype(bf16),
        "w2": W2.astype(bf16),
        "w3": W3.astype(bf16),
        "b1c": b1.reshape(H, 1).astype(np.float32),
        "b2r": np.broadcast_to(b2, (P, H)).copy().astype(np.float32),
        "b2r4": np.broadcast_to(np.tile(b2, 4), (P, 4 * H)).copy().astype(
            np.float32),
        "b3r": np.broadcast_to(b3, (P, H)).copy().astype(np.float32),
        "iotac": np.arange(P, dtype=np.float32).reshape(P, 1).astype(bf16),
        "iotar": np.broadcast_to(
            np.arange(P, dtype=np.float32), (P, P)).copy().astype(bf16),
        "zeros": np.zeros((P, 192), dtype=bf16),
    }
    return consts


# ---------------------------------------------------------------- device IR

def _build(struct, n_nodes):
    import concourse.bass as bass
    import concourse.mybir as mybir
    import concourse.tile as tile
    from concourse import bacc
    from concourse.tile_rust import add_dep_helper

    nw = struct["nw"]
    npc_pad = struct["npc_pad"]
    e_pad = struct["e_pad"]
    twg = struct["twg"]
    sw = struct["sw"]
    cmin = struct["cmin"]
    sw_max = int(max(int(s) for s in sw))

    bf = mybir.dt.bfloat16
    f32 = mybir.dt.float32
    AF = mybir.ActivationFunctionType
    ALU = mybir.AluOpType

    nc = bacc.Bacc("TRN2", target_bir_lowering=False)

    qidx = nc.dram_tensor("qidx", [P, e_pad // 16], mybir.dt.int16,
                          kind="ExternalInput")
    dstc = nc.dram_tensor("dstc", [P, e_pad // P], bf, kind="ExternalInput")
    dstr = nc.dram_tensor("dstr", [P, e_pad], bf, kind="ExternalInput")
    attrT = nc.dram_tensor("attrT", [ED, e_pad], bf, kind="ExternalInput")
    xq = nc.dram_tensor("xq", [n_nodes // 4, 256], bf, kind="ExternalInput")
    xTc = nc.dram_tensor("xTc", [H, npc_pad], bf, kind="ExternalInput")
    w1a2 = nc.dram_tensor("w1a2", [P, H], bf, kind="ExternalInput")
    w1b = nc.dram_tensor("w1b", [H, H], bf, kind="ExternalInput")
    w1c = nc.dram_tensor("w1c", [ED, H], bf, kind="ExternalInput")
    w2 = nc.dram_tensor("w2", [H, H], bf, kind="ExternalInput")
    w3 = nc.dram_tensor("w3", [P, H], bf, kind="ExternalInput")
    b1c = nc.dram_tensor("b1c", [H, 1], f32, kind="ExternalInput")
    b2r = nc.dram_tensor("b2r", [P, H], f32, kind="ExternalInput")
    b2r4 = nc.dram_tensor("b2r4", [P, 4 * H], f32, kind="ExternalInput")
    b3r = nc.dram_tensor("b3r", [P, H], f32, kind="ExternalInput")
    iotac = nc.dram_tensor("iotac", [P, 1], bf, kind="ExternalInput")
    iotar = nc.dram_tensor("iotar", [P, P], bf, kind="ExternalInput")
    zeros = nc.dram_tensor("zeros", [P, 192], bf, kind="ExternalInput")
    cnt = nc.dram_tensor("cnt", [1, nw * 4], mybir.dt.int32,
                         kind="ExternalInput")
    out = nc.dram_tensor("out", [npc_pad, H], f32, kind="ExternalOutput")

    with tile.TileContext(nc) as tc:
        with (
            tc.tile_pool(name="const", bufs=1) as cp,
            tc.tile_pool(name="gat", bufs=(6 if sw_max <= 2816 else 3)) as gp,
            tc.tile_pool(name="win", bufs=2) as wp,
            tc.tile_pool(name="work", bufs=3) as kp,
            tc.tile_pool(name="nodein", bufs=2) as np_,
            tc.tile_pool(name="outp", bufs=2) as op_,
            tc.tile_pool(name="ps_h", bufs=2, space="PSUM") as ph,
            tc.tile_pool(name="ps_m", bufs=2, space="PSUM") as pm,
            tc.tile_pool(name="ps_a", bufs=2, space="PSUM") as pa,
            tc.tile_pool(name="ps_x", bufs=1, space="PSUM") as px,
        ):
            def load_const(t, shape, dt):
                s = cp.tile(shape, dt, tag=t.name)
                nc.sync.dma_start(out=s[:], in_=t[:])
                return s

            w1a2t = load_const(w1a2, [P, H], bf)
            w1bt = load_const(w1b, [H, H], bf)
            w1ct = load_const(w1c, [ED, H], bf)
            w2t = load_const(w2, [H, H], bf)
            w3t = load_const(w3, [P, H], bf)
            b1t = load_const(b1c, [H, 1], f32)
            b2t = load_const(b2r, [P, H], f32)
            b2t4 = load_const(b2r4, [P, 4 * H], f32)
            b3t = load_const(b3r, [P, H], f32)
            iocat = load_const(iotac, [P, 1], bf)
            iorat = load_const(iotar, [P, P], bf)
            zt = load_const(zeros, [P, 192], bf)
            cntt = load_const(cnt, [1, nw * 4], mybir.dt.int32)

            prev_gather = None
            col16 = 0   # column offset into qidx (units of 16 idxs)
            colT = 0    # tile offset (units of 128 slots)

            for w in range(nw):
                s_w = int(sw[w])
                t_w = s_w // P

                gts = [None] * 4
                gwin = None
                if t_w and not USE_REG_COUNTS:
                    # one gather for the whole window; all pad idxs valid
                    idxt = wp.tile([P, s_w // 16], mybir.dt.int16, tag="idxt")
                    nc.sync.dma_start(
                        out=idxt[:], in_=qidx[:, col16:col16 + s_w // 16])
                    gwin = gp.tile([P, 2 * s_w], bf, tag="gat")
                    nc.gpsimd.dma_gather(
                        gwin[:].rearrange("p (b n) -> p b n", b=2),
                        xq[:], idxt[:], s_w, s_w, 256,
                        transpose=True, single_packet=False,
                    )
                elif t_w:
                    idxt = wp.tile([P, s_w // 16], mybir.dt.int16, tag="idxt")
                    nc.sync.dma_start(
                        out=idxt[:], in_=qidx[:, col16:col16 + s_w // 16])
                    goff16 = 0
                    for gi in range(4):
                        tg = int(twg[w][gi])
                        if tg == 0:
                            continue
                        ni = tg * P
                        gt = gp.tile([P, 2 * ni], bf, tag="gat")
                        c0 = (int(cmin[w][gi]) // 16) * 16
                        if c0 < ni:
                            # reg-count gathers skip trailing pad columns;
                            # zero them so 0*sel stays finite downstream
                            nc.vector.memset(
                                gt[:].rearrange("p (b n) -> p b n", b=2)
                                [:, :, c0:], 0)
                        if USE_REG_COUNTS:
                            bb = nc.cur_bb.bb
                            n0 = len(bb.instructions)
                            creg = nc.gpsimd.value_load(
                                cntt[0:1, w * 4 + gi:w * 4 + gi + 1],
                                min_val=1, max_val=ni)
                            load_insts = list(bb.instructions[n0:])
                        else:
                            creg = ni
                        gth = nc.gpsimd.dma_gather(
                            gt[:].rearrange("p (b n) -> p b n", b=2),
                            xq[:],
                            idxt[:, goff16:goff16 + ni // 16],
                            ni, creg, 256,
                            transpose=True, single_packet=False,
                        )
                        if USE_REG_COUNTS:
                            # chain the count reg-load behind the previous
                            # gather so only ~1 count register is live at a
                            # time (Pool has ~54 allocatable registers)
                            if prev_gather is not None:
                                for li in load_insts:
                                    add_dep_helper(
                                        li, prev_gather.ins, False,
                                        "serialize count reg-loads")
                            prev_gather = gth
                        gts[gi] = gt
                        goff16 += ni // 16
                if t_w:
                    dstct = wp.tile([P, t_w], bf, tag="dstct")
                    nc.sync.dma_start(
                        out=dstct[:], in_=dstc[:, colT:colT + t_w])
                    dstrt = wp.tile([P, s_w], bf, tag="dstrt")
                    nc.sync.dma_start(
                        out=dstrt[:], in_=dstr[:, colT * P:colT * P + s_w])
                    attrt = wp.tile([ED, s_w], bf, tag="attrt")
                    nc.sync.dma_start(
                        out=attrt[:], in_=attrT[:, colT * P:colT * P + s_w])

                # node window features -> node_in rows 0:64
                nit = np_.tile([P, P], bf, tag="nit")
                nc.sync.dma_start(
                    out=nit[0:H, :], in_=xTc[:, w * P:(w + 1) * P])

                # P2_win = x_win @ W1b   [128n, 64]
                p2ps = px.tile([P, H], f32, tag="p2ps")
                nc.tensor.matmul(p2ps[:], lhsT=nit[0:H, :], rhs=w1bt[:],
                                 start=True, stop=True)
                p2t = kp.tile([P, H], bf, tag="p2t")
                nc.vector.tensor_copy(out=p2t[:], in_=p2ps[:])

                # sel_T for the whole window, built in 512-wide chunks
                if t_w:
                    selT = wp.tile([P, s_w], bf, tag="selT")
                    for c0 in range(0, s_w, 512):
                        cw = min(512, s_w - c0)
                        nc.vector.tensor_tensor(
                            out=selT[:, c0:c0 + cw],
                            in0=iocat[:].to_broadcast([P, cw]),
                            in1=dstrt[:, c0:c0 + cw],
                            op=ALU.is_equal,
                        )

                aggps = pa.tile([H, P], f32, tag="aggps")
                tt = 0
                n_tiles = t_w
                for gi in range(4):
                    hb = 64 * (gi % 2)
                    blk = gi // 2
                    tg = int(twg[w][gi])
                    for tl in range(tg):
                        cols = slice(tt * P, (tt + 1) * P)
                        # h_T psum [64, 128]
                        hps = ph.tile([H, P], f32, tag="hps")
                        if gwin is not None:
                            xsrc_rhs = gwin[hb:hb + H,
                                            blk * s_w + tt * P:
                                            blk * s_w + (tt + 1) * P]
                        else:
                            xsrc_rhs = gts[gi][hb:hb + H,
                                               blk * tg * P + tl * P:
                                               blk * tg * P + (tl + 1) * P]
                        nc.tensor.matmul(
                            hps[:],
                            lhsT=w1a2t[hb:hb + H, :],
                            rhs=xsrc_rhs,
                            start=True, stop=False, skip_group_check=True)
                        nc.tensor.matmul(
                            hps[:], lhsT=w1ct[:], rhs=attrt[:, cols],
                            start=False, stop=False, skip_group_check=True)
                        nc.tensor.matmul(
                            hps[:], lhsT=p2t[:], rhs=selT[:, cols],
                            start=False, stop=True, skip_group_check=True)
                        hsb = kp.tile([H, P], bf, tag="hsb")
                        if USE_NATIVE_SILU:
                            nc.scalar.activation(hsb[:], hps[:], AF.Silu,
                                                 bias=b1t[:])
                        else:
                            zh = kp.tile([H, P], f32, tag="zh")
                            nc.vector.tensor_tensor(
                                out=zh[:], in0=hps[:],
                                in1=b1t[:].to_broadcast([H, P]), op=ALU.add)
                            sgh = kp.tile([H, P], f32, tag="sgh")
                            nc.scalar.activation(sgh[:], zh[:], AF.Sigmoid)
                            nc.vector.tensor_tensor(
                                out=hsb[:], in0=zh[:], in1=sgh[:],
                                op=ALU.mult)
                        # messages [128e, 64]
                        msgps = pm.tile([P, H], f32, tag="msgps")
                        nc.tensor.matmul(msgps[:], lhsT=hsb[:], rhs=w2t[:],
                                         start=True, stop=True,
                                         skip_group_check=True)
                        zt2 = kp.tile([P, H], f32, tag="zt2")
                        nc.vector.tensor_tensor(out=zt2[:], in0=msgps[:],
                                                in1=b2t[:], op=ALU.add)
                        msgt = kp.tile([P, H], bf, tag="msgt")
                        if USE_NATIVE_SILU:
                            nc.scalar.activation(msgt[:], zt2[:], AF.Silu)
                        else:
                            sgm = kp.tile([P, H], f32, tag="sgm")
                            nc.scalar.activation(sgm[:], zt2[:], AF.Sigmoid)
                            nc.vector.tensor_tensor(
                                out=msgt[:], in0=zt2[:], in1=sgm[:],
                                op=ALU.mult)
                        # sel [128e, 128n]
                        selt = kp.tile([P, P], bf, tag="selt")
                        nc.vector.tensor_tensor(
                            out=selt[:],
                            in0=dstct[:, tt:tt + 1].to_broadcast([P, P]),
                            in1=iorat[:],
                            op=ALU.is_equal,
                        )
                        nc.tensor.matmul(
                            aggps[:], lhsT=msgt[:], rhs=selt[:],
                            start=(tt == 0), stop=(tt == n_tiles - 1),
                            skip_group_check=True)
                        tt += 1

                if n_tiles == 0:
                    nc.tensor.matmul(aggps[:], lhsT=zt[:, 0:H],
                                     rhs=zt[:, 64:64 + P],
                                     start=True, stop=True,
                                     skip_group_check=True)

                # node MLP
                nc.vector.tensor_copy(out=nit[H:2 * H, :], in_=aggps[:])
                ops = px.tile([P, H], f32, tag="ops")
                nc.tensor.matmul(ops[:], lhsT=nit[:], rhs=w3t[:],
                                 start=True, stop=True, skip_group_check=True)
                zo = kp.tile([P, H], f32, tag="zo")
                nc.vector.tensor_tensor(out=zo[:], in0=ops[:], in1=b3t[:],
                                        op=ALU.add)
                oo = op_.tile([P, H], f32, tag="oo")
                if USE_NATIVE_SILU:
                    nc.scalar.activation(oo[:], zo[:], AF.Silu)
                else:
                    sgo = kp.tile([P, H], f32, tag="sgo")
                    nc.scalar.activation(sgo[:], zo[:], AF.Sigmoid)
                    nc.vector.tensor_tensor(out=oo[:], in0=zo[:], in1=sgo[:],
                                            op=ALU.mult)
                nc.sync.dma_start(out=out[w * P:(w + 1) * P, :], in_=oo[:])

                col16 += s_w // 16
                colT += t_w

    nc.compile()
    return nc


# ---------------------------------------------------------------- entry

def kernel(x, edge_index, edge_attr, W1, b1, W2, b2, W3, b3):
    import time
    t0 = time.time()
    x = np.asarray(x, dtype=np.float32)
    edge_index = np.asarray(edge_index)
    edge_attr = np.asarray(edge_attr, dtype=np.float32)

    struct, arrays = _prep(x, edge_index, edge_attr)
    consts = _prep_consts(
        np.asarray(W1, np.float32), np.asarray(b1, np.float32),
        np.asarray(W2, np.float32), np.asarray(b2, np.float32),
        np.asarray(W3, np.float32), np.asarray(b3, np.float32))
    t1 = time.time()

    nc = _build(struct, x.shape[0])
    t2 = time.time()
    print(f"[kernel] prep {t1 - t0:.1f}s  build+tile {t2 - t1:.1f}s")

    from concourse.bass_utils import run_bass_kernel_spmd
    in_maps = []
    for c in range(N_CORES):
        m = {
            "qidx": arrays["qidx"][c], "dstc": arrays["dstc"][c],
            "dstr": arrays["dstr"][c], "attrT": arrays["attrT"][c],
            "xq": arrays["xq"], "xTc": arrays["xTc"][c],
            "cnt": arrays["cnt"][c].reshape(1, -1),
        }
        m.update(consts)
        in_maps.append(m)
    t3 = time.time()
    res = run_bass_kernel_spmd(nc, in_maps, core_ids=list(range(N_CORES)))
    print(f"[kernel] compile+run {time.time() - t3:.1f}s")
    npc = struct["npc"]
    pieces = [res.results[c]["out"][:npc] for c in range(N_CORES)]
    return np.concatenate(pieces, axis=0).astype(np.float32)



# revision 3
# speedup vs baseline: 2.9144x; 2.9144x over previous
"""Trainium2 Bass kernel for a GNN message-passing layer.

Strategy (node-range sharding, host-side gather, no collectives):
  - 8 cores, each owns 12500 destination nodes (98 windows of 128 nodes).
  - Host sorts edges by (core, dst-window), pads each window to 128-edge
    tiles (tile counts = max over cores so one NEFF fits all cores), and
    pre-gathers x[src], x[dst] into an edge-major bf16 stream
    xsd[128, e_pad] (rows 0:64 = x[src]^T, rows 64:128 = x[dst]^T) plus
    attrA[17, e_pad] (edge_attr^T with a constant ones row for bias fold).
  - Device, per 512-edge chunk: h^T = W1^T m_in via 2 wide matmuls
    (K=128 for [xsrc;xdst], K=17 for [attr;1]); b1 folded into the attr
    weights; an extra output column makes silu produce a constant 1.0 row
    so b2 folds into the msg matmul. Then per 128-edge tile: msg
    edge-major via lhsT=h-slice, scatter-add to agg[64, 128n] in PSUM via
    a one-hot sel matmul. sel for the whole chunk is built in one DVE
    is_equal with a stride-0 broadcast AP.
  - Node MLP per window: out^T[64, 128n] = W3^T [x_win; agg] with b3 via
    the activation bias port; output written feat-major, host transposes.

All matmuls bf16 (f32 PSUM accumulate).
"""

import numpy as np
import ml_dtypes

P = 128
H = 64
ED = 16
N_CORES = 8
CHUNK = 4          # tiles per chunk (4*128 = 512 edges, one PSUM bank)


# ---------------------------------------------------------------- host prep

def _silu_inv_one():
    """z with z*sigmoid(z) == 1 (float64 Newton)."""
    z = 1.3
    for _ in range(50):
        s = 1.0 / (1.0 + np.exp(-z))
        f = z * s - 1.0
        df = s * (1.0 + z * (1.0 - s))
        z -= f / df
    return z


def _prep(x, edge_index, edge_attr):
    """Sort/pad edges into per-core slot layout; host-gather x[src]/x[dst]."""
    bf16 = ml_dtypes.bfloat16
    n_nodes = x.shape[0]
    npc = n_nodes // N_CORES              # nodes per core
    nw = (npc + P - 1) // P               # windows per core
    npc_pad = nw * P

    src = edge_index[0].astype(np.int64)
    dst = edge_index[1].astype(np.int64)
    e = src.shape[0]

    core = dst // npc
    rem = dst - core * npc
    wl = rem // P                         # window within core
    dloc = rem - wl * P                   # dst index within window

    key = (core * nw + wl).astype(np.int64)
    order = np.argsort(key, kind="stable")
    key_s = key[order]
    src_s = src[order]
    dst_s = dst[order]
    dloc_s = dloc[order]

    counts = np.bincount(key_s, minlength=N_CORES * nw).reshape(N_CORES, nw)
    tw = np.maximum((counts.max(axis=0) + P - 1) // P, 1)   # tiles per window
    sw = tw * P                                             # slots per window
    e_pad = int(sw.sum())
    t_tot = int(tw.sum())
    base = np.concatenate([[0], np.cumsum(sw)[:-1]])        # slot base per w

    # rank of each sorted edge within its (core, w) block
    starts = np.concatenate([[0], np.cumsum(counts.reshape(-1))[:-1]])
    rank = np.arange(e, dtype=np.int64) - starts[key_s]
    core_s = key_s // nw
    w_s = key_s - core_s * nw
    slot = base[w_s] + rank

    # edge-major streams (pad cols stay 0 / dloc -1)
    xsd = np.zeros((N_CORES, 2 * H, e_pad), dtype=bf16)
    xb = x.astype(bf16)
    xsd[core_s, :, slot] = np.concatenate([xb[src_s], xb[dst_s]], axis=1)

    attrA = np.zeros((N_CORES, ED + 1, e_pad), dtype=bf16)
    attrA[:, ED, :] = bf16(1.0)
    attrA[core_s, :, slot] = np.concatenate(
        [edge_attr[order].astype(bf16),
         np.ones((e, 1), dtype=bf16)], axis=1)

    dloc_slots = np.full((N_CORES, e_pad), -1.0, dtype=np.float32)
    dloc_slots[core_s, slot] = dloc_s.astype(np.float32)
    dstc = np.ascontiguousarray(
        dloc_slots.reshape(N_CORES, t_tot, P).transpose(0, 2, 1)
    ).astype(bf16)                                          # [C, 128, Ttot]

    xT = x.T.astype(bf16)                                   # [64, N]
    xTn = np.zeros((N_CORES, H, npc_pad), dtype=bf16)
    for c in range(N_CORES):
        xTn[c, :, :npc] = xT[:, c * npc:(c + 1) * npc]

    struct = {"nw": nw, "npc": npc, "npc_pad": npc_pad, "e_pad": e_pad,
              "t_tot": t_tot, "tw": tw, "sw": sw}
    arrays = {"xsd": xsd, "attrA": attrA, "dstc": dstc, "xTn": xTn}
    return struct, arrays


def _prep_consts(W1, b1, W2, b2, W3, b3):
    bf16 = ml_dtypes.bfloat16
    z1 = _silu_inv_one()

    w1ab = np.zeros((2 * H, H + 1), dtype=bf16)
    w1ab[:, :H] = W1[0:2 * H, :].astype(bf16)

    w1ca = np.zeros((ED + 1, H + 1), dtype=bf16)
    w1ca[0:ED, :H] = W1[2 * H:2 * H + ED, :].astype(bf16)
    w1ca[ED, :H] = b1.astype(bf16)
    w1ca[ED, H] = bf16(z1)          # silu -> exact-ish 1.0 constant row

    w2a = np.zeros((H + 1, H), dtype=bf16)
    w2a[0:H, :] = W2.astype(bf16)
    w2a[H, :] = b2.astype(bf16)

    iorat4 = np.broadcast_to(
        np.tile(np.arange(P, dtype=np.float32), CHUNK), (P, CHUNK * P)
    ).copy().astype(bf16)

    consts = {
        "w1ab": w1ab,
        "w1ca": w1ca,
        "w2a": w2a,
        "w3": W3.astype(bf16),
        "b3c": b3.reshape(H, 1).astype(np.float32),
        "iorat4": iorat4,
        "zeros64": np.zeros((H, P), dtype=bf16),
    }
    return consts


# ---------------------------------------------------------------- device IR

def _build(struct):
    import concourse.mybir as mybir
    import concourse.tile as tile
    from concourse import bacc

    nw = struct["nw"]
    npc_pad = struct["npc_pad"]
    e_pad = struct["e_pad"]
    t_tot = struct["t_tot"]
    tw = struct["tw"]

    bf = mybir.dt.bfloat16
    f32 = mybir.dt.float32
    AF = mybir.ActivationFunctionType
    ALU = mybir.AluOpType

    nc = bacc.Bacc("TRN2", target_bir_lowering=False)

    xsd = nc.dram_tensor("xsd", [2 * H, e_pad], bf, kind="ExternalInput")
    attrA = nc.dram_tensor("attrA", [ED + 1, e_pad], bf, kind="ExternalInput")
    dstc = nc.dram_tensor("dstc", [P, t_tot], bf, kind="ExternalInput")
    xTn = nc.dram_tensor("xTn", [H, npc_pad], bf, kind="ExternalInput")
    w1ab = nc.dram_tensor("w1ab", [2 * H, H + 1], bf, kind="ExternalInput")
    w1ca = nc.dram_tensor("w1ca", [ED + 1, H + 1], bf, kind="ExternalInput")
    w2a = nc.dram_tensor("w2a", [H + 1, H], bf, kind="ExternalInput")
    w3 = nc.dram_tensor("w3", [2 * H, H], bf, kind="ExternalInput")
    b3c = nc.dram_tensor("b3c", [H, 1], f32, kind="ExternalInput")
    iorat4 = nc.dram_tensor("iorat4", [P, CHUNK * P], bf, kind="ExternalInput")
    zeros64 = nc.dram_tensor("zeros64", [H, P], bf, kind="ExternalInput")
    outT = nc.dram_tensor("outT", [H, npc_pad], f32, kind="ExternalOutput")

    with tile.TileContext(nc) as tc:
        with (
            tc.tile_pool(name="const", bufs=1) as cp,
            tc.tile_pool(name="win", bufs=2) as wp,
            tc.tile_pool(name="work", bufs=3) as kp,
            tc.tile_pool(name="nodein", bufs=2) as np_,
            tc.tile_pool(name="outp", bufs=2) as op_,
            tc.tile_pool(name="ps_h", bufs=2, space="PSUM") as ph,
            tc.tile_pool(name="ps_m", bufs=2, space="PSUM") as pm,
            tc.tile_pool(name="ps_a", bufs=2, space="PSUM") as pa,
            tc.tile_pool(name="ps_x", bufs=2, space="PSUM") as px,
        ):
            def load_const(t, shape, dt):
                s = cp.tile(shape, dt, tag=t.name)
                nc.sync.dma_start(out=s[:], in_=t[:])
                return s

            w1abt = load_const(w1ab, [2 * H, H + 1], bf)
            w1cat = load_const(w1ca, [ED + 1, H + 1], bf)
            w2at = load_const(w2a, [H + 1, H], bf)
            w3t = load_const(w3, [2 * H, H], bf)
            b3t = load_const(b3c, [H, 1], f32)
            iot = load_const(iorat4, [P, CHUNK * P], bf)
            zt = load_const(zeros64, [H, P], bf)

            col = 0    # slot offset
            colT = 0   # tile offset

            for w in range(nw):
                t_w = int(tw[w])
                s_w = t_w * P

                tA = wp.tile([2 * H, s_w], bf, tag="tA")
                nc.sync.dma_start(out=tA[:], in_=xsd[:, col:col + s_w])
                tB = wp.tile([ED + 1, s_w], bf, tag="tB")
                nc.sync.dma_start(out=tB[:], in_=attrA[:, col:col + s_w])
                dct = wp.tile([P, t_w], bf, tag="dct")
                nc.sync.dma_start(out=dct[:], in_=dstc[:, colT:colT + t_w])
                nit = np_.tile([P, P], bf, tag="nit")
                nc.sync.dma_start(out=nit[0:H, :],
                                  in_=xTn[:, w * P:(w + 1) * P])

                aggps = pa.tile([H, P], f32, tag="agg")

                tt = 0
                for c0 in range(0, t_w, CHUNK):
                    tpc = min(CHUNK, t_w - c0)
                    cw = tpc * P
                    cols = slice(c0 * P, c0 * P + cw)

                    hps = ph.tile([H + 1, CHUNK * P], f32, tag="hps")
                    nc.tensor.matmul(hps[:, :cw], lhsT=w1abt[:],
                                     rhs=tA[:, cols],
                                     start=True, stop=False,
                                     skip_group_check=True)
                    nc.tensor.matmul(hps[:, :cw], lhsT=w1cat[:],
                                     rhs=tB[:, cols],
                                     start=False, stop=True,
                                     skip_group_check=True)
                    hsb = kp.tile([H + 1, CHUNK * P], bf, tag="hsb")
                    nc.scalar.activation(hsb[:, :cw], hps[:, :cw], AF.Silu)

                    # one-hot sel for the whole chunk: sel[p, t, n] =
                    # (dloc[tile t, edge p] == n)
                    selc = kp.tile([P, CHUNK * P], bf, tag="selc")
                    nc.vector.tensor_tensor(
                        out=selc[:, :cw].rearrange("p (c o) -> p c o", o=P),
                        in0=dct[:, c0:c0 + tpc]
                            .rearrange("p (c o) -> p c o", o=1)
                            .to_broadcast([P, tpc, P]),
                        in1=iot[:, :cw].rearrange("p (c o) -> p c o", o=P),
                        op=ALU.is_equal,
                    )

                    msgps = pm.tile([P, CHUNK * H], f32, tag="msgps")
                    for t in range(tpc):
                        nc.tensor.matmul(
                            msgps[:, t * H:(t + 1) * H],
                            lhsT=hsb[:, t * P:(t + 1) * P],
                            rhs=w2at[:],
                            start=True, stop=True, skip_group_check=True)
                    msgt = kp.tile([P, CHUNK * H], bf, tag="msgt")
                    nc.scalar.activation(msgt[:, :tpc * H],
                                         msgps[:, :tpc * H], AF.Silu)

                    for t in range(tpc):
                        nc.tensor.matmul(
                            aggps[:],
                            lhsT=msgt[:, t * H:(t + 1) * H],
                            rhs=selc[:, t * P:(t + 1) * P],
                            start=(tt == 0), stop=(tt == t_w - 1),
                            skip_group_check=True)
                        tt += 1

                # node MLP (feat-major): out^T = silu(W3^T [x_win; agg] + b3)
                nc.vector.tensor_copy(out=nit[H:2 * H, :], in_=aggps[:])
                ops = px.tile([H, P], f32, tag="ops")
                nc.tensor.matmul(ops[:], lhsT=w3t[:], rhs=nit[:],
                                 start=True, stop=True, skip_group_check=True)
                oo = op_.tile([H, P], f32, tag="oo")
                nc.scalar.activation(oo[:], ops[:], AF.Silu, bias=b3t[:])
                nc.sync.dma_start(out=outT[:, w * P:(w + 1) * P], in_=oo[:])

                col += s_w
                colT += t_w

    nc.compile()
    return nc


# ---------------------------------------------------------------- entry

def kernel(x, edge_index, edge_attr, W1, b1, W2, b2, W3, b3):
    import time
    t0 = time.time()
    x = np.asarray(x, dtype=np.float32)
    edge_index = np.asarray(edge_index)
    edge_attr = np.asarray(edge_attr, dtype=np.float32)

    struct, arrays = _prep(x, edge_index, edge_attr)
    consts = _prep_consts(
        np.asarray(W1, np.float32), np.asarray(b1, np.float32),
        np.asarray(W2, np.float32), np.asarray(b2, np.float32),
        np.asarray(W3, np.float32), np.asarray(b3, np.float32))
    t1 = time.time()

    nc = _build(struct)
    t2 = time.time()
    print(f"[kernel] prep {t1 - t0:.1f}s  build+tile {t2 - t1:.1f}s")

    from concourse.bass_utils import run_bass_kernel_spmd
    in_maps = []
    for c in range(N_CORES):
        m = {
            "xsd": arrays["xsd"][c], "attrA": arrays["attrA"][c],
            "dstc": arrays["dstc"][c], "xTn": arrays["xTn"][c],
        }
        m.update(consts)
        in_maps.append(m)
    t3 = time.time()
    res = run_bass_kernel_spmd(nc, in_maps, core_ids=list(range(N_CORES)))
    print(f"[kernel] compile+run {time.time() - t3:.1f}s")
    npc = struct["npc"]
    pieces = [np.ascontiguousarray(res.results[c]["outT"][:, :npc].T)
              for c in range(N_CORES)]
    return np.concatenate(pieces, axis=0).astype(np.float32)
